# revision 23
# baseline (speedup 1.0000x reference)
"""GNN message-passing layer on 8 Trainium2 NeuronCores.

Receiver-range sharding (core c owns nodes [c*12672, (c+1)*12672)); no
cross-core collectives.  v5: 96-node windows with a fused per-tile matmul.

The gather table holds P_s = nodes @ W1_send (host-precomputed per-node
projection), so gathered rows arrive already in [edge, hidden] layout (no
DMA transpose).  Windows are 96 nodes so the receiver-selection matrix
maskT (96 rows) stacks with the 32 edge-feature rows into ONE fused fp8
stationary [128, 128e]; the moving operand stacks PR' = nodes@W1_recv + b1
(96 rows, per window) with W1_edge (32 rows).  One matmul per 128-edge tile
computes inject+edge contribution; em (gathered P_s) is added on DVE; relu
on ACT; one matmul per tile aggregates h into aggT via the fp8 0/1 mask.

Host-side layout per core:
  - nodes padded to 101376 = 1056 windows of 96; each core owns 132 windows
  - per (window, quarter) group: first capq=384 edges (sender-quarter-local
    ids fit dma_gather's int16 limit); excess edges spill to a per-window
    overflow group (capacity capo) fed from table rows [101376, +25600)
  - slot order: [batch of B windows][group 0..4][window-in-batch][cap]
  - fs  fp8 [128, slots]: rows 0:96 maskT (fs[n, s] = recv(s)==n), rows
    96:128 edge features
  - mask fp8 [128, 96*tiles]: mask[s&127, (s>>7)*96 + n] = (recv(s)==n)
  - prd bf16 [128, W*128]: rows 0:96 PR' per window, rows 96:128 W1_edge
"""
import numpy as np
import ml_dtypes
from contextlib import ExitStack

import concourse.bass as bass
import concourse.tile as tile
from concourse import bacc, mybir
import concourse.bass_utils as bass_utils

BF16 = mybir.dt.bfloat16
F32 = mybir.dt.float32
I16 = mybir.dt.int16
FP8 = mybir.dt.float8e4
bfnp = ml_dtypes.bfloat16
f8np = ml_dtypes.float8_e4m3

# problem shapes (hardcoded per harness contract)
N_NODES = 100000
N_EDGES = 1600000
NODE_F = 64
EDGE_F = 32
OUT_F = 64
HIDDEN = 128

NCORES = 8
WN = 96                       # nodes per window (96 + EDGE_F = 128)
W_CORE = 132                  # windows per core
W_TOTAL = W_CORE * NCORES     # 1056
NODES_PAD = W_TOTAL * WN      # 101376
NODES_CORE = W_CORE * WN      # 12672
QUARTER = NODES_PAD // 4      # 25344 (< int16 max)
OVR = 25600                   # overflow table region rows
B = 4                         # windows per batch
NBATCH = W_CORE // B          # 33

_cache = {}


def _build_program(capq: int, capo: int):
    """Build + compile the (single, SPMD-shared) Bass program."""
    caps = [capq] * 4 + [capo]
    slots_w = sum(caps)               # slots per window
    slots_b = B * slots_w             # slots per batch
    slots_core = W_CORE * slots_w
    tiles_b = slots_b // 128
    tiles_core = slots_core // 128
    # batch-local offset of each group's region
    goff = [0]
    for c in caps:
        goff.append(goff[-1] + B * c)
    # group pairs share one PSUM tile so the em-add / relu run as wide ops
    pairs = [(0, 1), (2, 3), (4,)]
    pw = max(2 * capq, capo)          # pair tile width

    # NOTE: 16KB descriptor carveout (1024-desc SWDGE rings) measured faster
    # than 32/64KB -- larger rings let gather descriptors swamp the DMA
    # engines and starve the dense loads compute needs next
    nc = bacc.Bacc("TRN2", target_bir_lowering=False, debug=False,
                   enable_asserts=False, num_devices=NCORES,
                   num_swdge_queues=4, dynamic_dma_scratch_size=16384)

    tbl_s = nc.dram_tensor("tbl_s", [NODES_PAD + OVR, 128], BF16,
                           kind="ExternalInput")
    fsd = nc.dram_tensor("fsd", [128, slots_core], FP8, kind="ExternalInput")
    maskd = nc.dram_tensor("maskd", [128, tiles_core * WN], FP8,
                           kind="ExternalInput")
    sidx = nc.dram_tensor("sidx", [128, slots_core // 16], I16,
                          kind="ExternalInput")
    prd = nc.dram_tensor("prd", [128, W_CORE * 128], BF16,
                         kind="ExternalInput")
    nodes_t = nc.dram_tensor("nodes_t", [66, NODES_CORE], F32,
                             kind="ExternalInput")
    invdeg = nc.dram_tensor("invdeg", [WN, W_CORE], F32, kind="ExternalInput")
    w2 = nc.dram_tensor("w2", [HIDDEN, OUT_F], BF16, kind="ExternalInput")
    waug = nc.dram_tensor("waug", [66, OUT_F], F32, kind="ExternalInput")
    out_d = nc.dram_tensor("out", [NODES_CORE, OUT_F], F32,
                           kind="ExternalOutput")

    relu = mybir.ActivationFunctionType.Relu
    cpy = mybir.ActivationFunctionType.Copy

    with tile.TileContext(nc) as tc:
        with ExitStack() as ctx:
            cpool = ctx.enter_context(tc.tile_pool(name="const", bufs=1))
            bpool = ctx.enter_context(tc.tile_pool(name="batch", bufs=3))
            empool = ctx.enter_context(tc.tile_pool(name="emp", bufs=4))
            spool = ctx.enter_context(tc.tile_pool(name="small", bufs=4))
            opool = ctx.enter_context(tc.tile_pool(name="outs", bufs=3))
            ph = ctx.enter_context(tc.tile_pool(name="ph", bufs=2, space="PSUM"))
            pagg = ctx.enter_context(tc.tile_pool(name="pagg", bufs=2,
                                                  space="PSUM"))
            pout = ctx.enter_context(tc.tile_pool(name="pout", bufs=1,
                                                  space="PSUM"))
            obpool = ctx.enter_context(tc.tile_pool(name="ob", bufs=2))

            w2_t = cpool.tile([HIDDEN, OUT_F], BF16)
            nc.sync.dma_start(w2_t[:], w2.ap())
            waug_t = cpool.tile([66, OUT_F], F32)
            nc.sync.dma_start(waug_t[:], waug.ap())
            invdeg_t = cpool.tile([WN, W_CORE], F32)
            nc.sync.dma_start(invdeg_t[:], invdeg.ap())
            sidx_t = cpool.tile([128, slots_core // 16], I16)
            nc.sync.dma_start(sidx_t[:], sidx.ap())

            for b in range(NBATCH):
                s0 = b * slots_b                      # batch slot base
                em = empool.tile([128, slots_b], BF16, tag="em")
                fs = bpool.tile([128, slots_b], FP8, tag="fs")
                mk = bpool.tile([128, tiles_b * WN], FP8, tag="mk")
                pr = bpool.tile([128, B * 128], BF16, tag="pr")
                nodesb_t = bpool.tile([66, B * WN], F32, tag="nodesb")
                nc.sync.dma_start(fs[:], fsd.ap()[:, s0:s0 + slots_b])
                nc.sync.dma_start(
                    mk[:], maskd.ap()[:, b * tiles_b * WN:(b + 1) * tiles_b * WN])
                nc.sync.dma_start(pr[:],
                                  prd.ap()[:, b * B * 128:(b + 1) * B * 128])
                nc.sync.dma_start(nodesb_t[:],
                                  nodes_t.ap()[:, b * B * WN:(b + 1) * B * WN])

                # sender gathers: groups 0-3 source quarters, group 4 the
                # per-core overflow region; spread over 4 SWDGE queues
                for g in range(5):
                    gs = B * caps[g]
                    base = g * QUARTER if g < 4 else NODES_PAD
                    nc.gpsimd.dma_gather(
                        out_ap=em[:, goff[g]:goff[g] + gs]
                        .rearrange("p (c f) -> p c f", f=128),
                        in_ap=tbl_s.ap()[base:base + (QUARTER if g < 4 else OVR), :],
                        idxs_ap=sidx_t[:, (s0 + goff[g]) // 16:
                                       (s0 + goff[g] + gs) // 16],
                        num_idxs=gs, num_idxs_reg=gs, elem_size=128,
                        transpose=False, single_packet=False,
                        queue_num=(g + b) % 4,
                    )

                otb = obpool.tile([WN, B * OUT_F], F32, tag="otb")
                for wi in range(B):
                    wg = b * B + wi                   # global window index
                    agg_ps = pagg.tile([128, WN], F32, tag="agg")
                    for pi, pair in enumerate(pairs):
                        h_ps = ph.tile([128, pw], F32, tag="h")
                        hp = spool.tile([128, pw], BF16, tag="hp")
                        pcols = sum(caps[g] for g in pair)
                        for gi, g in enumerate(pair):
                            cg = caps[g]
                            off = goff[g] + wi * cg   # batch-local slot base
                            for j in range(cg // 128):
                                so = off + j * 128
                                nc.tensor.matmul(
                                    out=h_ps[:, gi * capq + j * 128:
                                             gi * capq + (j + 1) * 128],
                                    lhsT=fs[:, so:so + 128],
                                    rhs=pr[:, wi * 128:(wi + 1) * 128],
                                    start=True, stop=True)
                        if len(pair) == 2:
                            # one wide add for both groups: em view [128,2,capq]
                            # strided by the group-region pitch B*capq
                            a = em[:, goff[pair[0]] + wi * capq:
                                   goff[pair[0]] + wi * capq + capq]
                            em3 = bass.AP(a.tensor, a.offset,
                                          [list(a.ap[0]), [B * capq, 2],
                                           [1, capq]])
                            nc.vector.tensor_add(
                                hp[:, :2 * capq]
                                .rearrange("p (c f) -> p c f", f=capq),
                                em3,
                                h_ps[:, :2 * capq]
                                .rearrange("p (c f) -> p c f", f=capq))
                        else:
                            cg = caps[pair[0]]
                            off = goff[pair[0]] + wi * cg
                            nc.vector.tensor_add(hp[:, :cg],
                                                 em[:, off:off + cg],
                                                 h_ps[:, :cg])
                        hs = spool.tile([128, pw], BF16, tag="hs")
                        nc.scalar.activation(hs[:, :pcols], hp[:, :pcols], relu)
                        for gi, g in enumerate(pair):
                            cg = caps[g]
                            off = goff[g] + wi * cg
                            for j in range(cg // 128):
                                tb = (off + j * 128) // 128   # batch tile idx
                                nc.tensor.matmul(
                                    out=agg_ps[:],
                                    lhsT=hs[:, gi * capq + j * 128:
                                            gi * capq + (j + 1) * 128],
                                    rhs=mk[:, tb * WN:(tb + 1) * WN],
                                    start=(pi == 0 and gi == 0 and j == 0),
                                    stop=(pi == 2 and j == cg // 128 - 1))
                    # window epilogue: out = (aggT.T@W2)*invdeg + nodes@waug
                    agg_s = opool.tile([128, WN], BF16, tag="aggs")
                    nc.scalar.activation(agg_s[:], agg_ps[:], cpy)
                    ot_ps = pout.tile([WN, 2 * OUT_F], F32, tag="ot")
                    nc.tensor.matmul(out=ot_ps[:, 0:OUT_F], lhsT=agg_s[:],
                                     rhs=w2_t[:], start=True, stop=True)
                    nc.tensor.matmul(out=ot_ps[:, OUT_F:2 * OUT_F],
                                     lhsT=nodesb_t[:, wi * WN:(wi + 1) * WN],
                                     rhs=waug_t[:], start=True, stop=True)
                    t1 = opool.tile([WN, OUT_F], F32, tag="t1")
                    nc.vector.tensor_scalar(
                        out=t1[:], in0=ot_ps[:, 0:OUT_F],
                        scalar1=invdeg_t[:, wg:wg + 1],
                        scalar2=None, op0=mybir.AluOpType.mult)
                    nc.vector.tensor_add(otb[:, wi * OUT_F:(wi + 1) * OUT_F],
                                         t1[:], ot_ps[:, OUT_F:2 * OUT_F])
                nc.sync.dma_start(
                    out_d.ap()[b * B * WN:(b + 1) * B * WN, :]
                    .rearrange("(w n) f -> n w f", n=WN),
                    otb[:].rearrange("p (w f) -> p w f", f=OUT_F))

    nc.compile()
    return nc


def _prep_inputs(nodes, edges, senders, receivers, W1, b1, W2, b2, Wn, bn,
                 capq, capo):
    """Host-side data layout. Returns per-core in_maps."""
    caps = [capq] * 4 + [capo]
    slots_w = sum(caps)
    slots_core = W_CORE * slots_w
    tiles_core = slots_core // 128
    goff = [0]
    for c in caps:
        goff.append(goff[-1] + B * c)
    slots_b = B * slots_w

    nodes_pad = np.zeros((NODES_PAD, NODE_F), np.float32)
    nodes_pad[:N_NODES] = nodes

    # per-node projections (host): sender table + receiver table (+b1)
    ps_full = (nodes_pad @ W1[:NODE_F]).astype(bfnp)            # [NP, 128]
    pr_full = (nodes_pad @ W1[NODE_F:2 * NODE_F] + b1).astype(bfnp)
    w1e = W1[2 * NODE_F:].astype(bfnp)                          # [32, 128]

    tbl_base = np.zeros((NODES_PAD + OVR, 128), bfnp)
    tbl_base[:NODES_PAD] = ps_full

    deg = np.bincount(receivers, minlength=NODES_PAD).astype(np.float32)
    invdeg_full = 1.0 / np.maximum(deg, 1.0)
    gate_full = (deg > 0).astype(np.float32)

    w2b = W2.astype(bfnp)
    waug = np.zeros((66, OUT_F), np.float32)
    waug[:NODE_F] = Wn
    waug[NODE_F] = b2
    waug[NODE_F + 1] = bn

    core_of_edge = receivers // NODES_CORE
    in_maps = []
    for c in range(NCORES):
        lo = c * NODES_CORE
        eid = np.nonzero(core_of_edge == c)[0]
        rloc = receivers[eid] - lo
        w_loc = rloc // WN
        q = senders[eid] // QUARTER
        order = np.lexsort((q, w_loc))
        eid, rloc, w_loc, q = eid[order], rloc[order], w_loc[order], q[order]
        grp = w_loc * 4 + q
        starts = np.searchsorted(grp, np.arange(W_CORE * 4))
        pos = np.arange(len(eid)) - starts[grp]

        w_batch = w_loc // B
        w_in = w_loc % B
        normal = pos < capq
        # normal slots: group q region of the window's batch
        goff_a = np.array(goff[:5])
        slot = np.empty(len(eid), np.int64)
        slot[normal] = (w_batch[normal] * slots_b + goff_a[q[normal]]
                        + w_in[normal] * capq + pos[normal])
        sidx_v = np.empty(len(eid), np.int64)
        sidx_v[normal] = senders[eid[normal]] % QUARTER

        # overflow: per-window list, region-5 table rows
        ov = ~normal
        ov_idx = np.nonzero(ov)[0]
        ovw = w_loc[ov_idx]
        ostarts = np.searchsorted(ovw, np.arange(W_CORE))
        opos = np.arange(len(ov_idx)) - ostarts[ovw]
        assert len(ov_idx) <= OVR, f"overflow {len(ov_idx)} > {OVR}"
        assert opos.max(initial=0) < capo, \
            f"window overflow {opos.max(initial=0) + 1} > {capo}"
        slot[ov_idx] = (w_batch[ov_idx] * slots_b + goff[4]
                        + w_in[ov_idx] * capo + opos)
        sidx_v[ov_idx] = np.arange(len(ov_idx))
        tbl_c = tbl_base.copy()
        tbl_c[NODES_PAD:NODES_PAD + len(ov_idx)] = \
            tbl_base[senders[eid[ov_idx]]]

        sidx_f = np.zeros(slots_core, np.int16)
        sidx_f[slot] = sidx_v.astype(np.int16)
        nloc = (rloc % WN).astype(np.int64)
        # fused stationary: rows 0:96 maskT, rows 96:128 edge features (fp8)
        fsd_c = np.zeros((128, slots_core), f8np)
        fsd_c[nloc, slot] = 1.0
        fsd_c[WN:, slot] = edges[eid].T.astype(f8np)
        # aggregation mask, 96 cols per 128-slot tile
        maskd_c = np.zeros((128, tiles_core * WN), f8np)
        maskd_c[slot & 127, (slot >> 7) * WN + nloc] = 1.0

        nodes_taug = np.zeros((66, NODES_CORE), np.float32)
        nodes_taug[:NODE_F] = nodes_pad[lo:lo + NODES_CORE].T
        nodes_taug[NODE_F] = gate_full[lo:lo + NODES_CORE]
        nodes_taug[NODE_F + 1] = 1.0

        # moving-operand table: rows 0:96 PR' per window, rows 96:128 W1e
        prd_c = np.zeros((128, W_CORE * 128), bfnp)
        prd_c[:WN] = (pr_full[lo:lo + NODES_CORE].reshape(W_CORE, WN, HIDDEN)
                      .transpose(1, 0, 2).reshape(WN, W_CORE * HIDDEN))
        prd_c[WN:] = np.tile(w1e, (1, W_CORE))

        in_maps.append({
            "tbl_s": tbl_c,
            "fsd": fsd_c,
            "maskd": maskd_c,
            "sidx": np.tile(sidx_f.reshape(-1, 16).T, (8, 1)),
            "prd": prd_c,
            "nodes_t": nodes_taug,
            "invdeg": invdeg_full[lo:lo + NODES_CORE].reshape(W_CORE, WN).T.copy(),
            "w2": w2b, "waug": waug,
        })
    return in_maps


def kernel(nodes, edges, senders, receivers, W1, b1, W2, b2, Wn, bn,
           _trace=False):
    senders = np.asarray(senders).astype(np.int64)
    receivers = np.asarray(receivers).astype(np.int64)
    nodes = np.asarray(nodes, np.float32)
    edges = np.asarray(edges, np.float32)

    # capacities; grow if the data exceeds them (recompile)
    capq, capo = 384, 128
    cw = (receivers // NODES_CORE) * (W_CORE * 4) \
        + ((receivers % NODES_CORE) // WN) * 4 + senders // QUARTER
    cnt = np.bincount(cw, minlength=NCORES * W_CORE * 4)
    ovw = np.maximum(cnt - capq, 0).reshape(-1, 4).sum(1)
    while ovw.max() > capo:
        capo += 128
    if np.maximum(cnt - capq, 0).reshape(NCORES, -1).sum(1).max() > OVR:
        capq += 128  # pathological: shrink overflow volume instead

    key = (capq, capo)
    if key not in _cache:
        _cache[key] = _build_program(capq, capo)
    nc = _cache[key]

    in_maps = _prep_inputs(nodes, edges, senders, receivers,
                           np.asarray(W1, np.float32), np.asarray(b1, np.float32),
                           np.asarray(W2, np.float32), np.asarray(b2, np.float32),
                           np.asarray(Wn, np.float32), np.asarray(bn, np.float32),
                           capq, capo)

    res = bass_utils.run_bass_kernel_spmd(
        nc, in_maps, core_ids=list(range(NCORES)), trace=_trace)

    out = np.concatenate([res.results[c]["out"] for c in range(NCORES)], axis=0)
    kernel.last_results = res
    return out[:N_NODES]


# revision 24
# speedup vs baseline: 1.4820x; 1.4820x over previous
"""GNN message-passing layer on 8 Trainium2 NeuronCores.

Receiver-range sharding (core c owns nodes [c*12672, (c+1)*12672)); no
cross-core collectives.  v5: 96-node windows with a fused per-tile matmul.

The gather table holds P_s = nodes @ W1_send (host-precomputed per-node
projection), so gathered rows arrive already in [edge, hidden] layout (no
DMA transpose).  Windows are 96 nodes so the receiver-selection matrix
maskT (96 rows) stacks with the 32 edge-feature rows into ONE fused fp8
stationary [128, 128e]; the moving operand stacks PR' = nodes@W1_recv + b1
(96 rows, per window) with W1_edge (32 rows).  One matmul per 128-edge tile
computes inject+edge contribution; em (gathered P_s) is added on DVE; relu
on ACT; one matmul per tile aggregates h into aggT via the fp8 0/1 mask.

Host-side layout per core:
  - nodes padded to 101376 = 1056 windows of 96; each core owns 132 windows
  - per (window, quarter) group: first capq=384 edges (sender-quarter-local
    ids fit dma_gather's int16 limit); excess edges spill to a per-window
    overflow group (capacity capo) fed from table rows [101376, +25600)
  - slot order: [batch of B windows][group 0..4][window-in-batch][cap]
  - fs  fp8 [128, slots]: rows 0:96 maskT (fs[n, s] = recv(s)==n), rows
    96:128 edge features
  - mask fp8 [128, 96*tiles]: mask[s&127, (s>>7)*96 + n] = (recv(s)==n)
  - prd bf16 [128, W*128]: rows 0:96 PR' per window, rows 96:128 W1_edge
"""
import numpy as np
import ml_dtypes
from contextlib import ExitStack

import concourse.bass as bass
import concourse.tile as tile
from concourse import bacc, mybir
import concourse.bass_utils as bass_utils

BF16 = mybir.dt.bfloat16
F32 = mybir.dt.float32
I16 = mybir.dt.int16
FP8 = mybir.dt.float8e4
bfnp = ml_dtypes.bfloat16
f8np = ml_dtypes.float8_e4m3

# problem shapes (hardcoded per harness contract)
N_NODES = 100000
N_EDGES = 1600000
NODE_F = 64
EDGE_F = 32
OUT_F = 64
HIDDEN = 128

NCORES = 8
WN = 96                       # nodes per window (96 + EDGE_F = 128)
W_CORE = 132                  # windows per core
W_TOTAL = W_CORE * NCORES     # 1056
NODES_PAD = W_TOTAL * WN      # 101376
NODES_CORE = W_CORE * WN      # 12672
QUARTER = NODES_PAD // 4      # 25344 (< int16 max)
OVR = 25600                   # overflow table region rows
B = 4                         # windows per batch
NBATCH = W_CORE // B          # 33

_cache = {}


def _build_program(capq: int, capo: int):
    """Build + compile the (single, SPMD-shared) Bass program."""
    caps = [capq] * 4 + [capo]
    slots_w = sum(caps)               # slots per window
    slots_b = B * slots_w             # slots per batch
    slots_core = W_CORE * slots_w
    tiles_b = slots_b // 128
    tiles_core = slots_core // 128
    # batch-local offset of each group's region
    goff = [0]
    for c in caps:
        goff.append(goff[-1] + B * c)
    # group pairs share one PSUM tile so the em-add / relu run as wide ops
    pairs = [(0, 1), (2, 3), (4,)]
    pw = max(2 * capq, capo)          # pair tile width

    # NOTE: 16KB descriptor carveout (1024-desc SWDGE rings) measured faster
    # than 32/64KB -- larger rings let gather descriptors swamp the DMA
    # engines and starve the dense loads compute needs next
    nc = bacc.Bacc("TRN2", target_bir_lowering=False, debug=False,
                   enable_asserts=False, num_devices=NCORES,
                   num_swdge_queues=4, dynamic_dma_scratch_size=16384)

    tbl_s = nc.dram_tensor("tbl_s", [NODES_PAD + W_CORE * capo, 128], BF16,
                           kind="ExternalInput")
    fsd = nc.dram_tensor("fsd", [128, slots_core], FP8, kind="ExternalInput")
    maskd = nc.dram_tensor("maskd", [128, tiles_core * WN], FP8,
                           kind="ExternalInput")
    sidx = nc.dram_tensor("sidx", [128, slots_core // 16], I16,
                          kind="ExternalInput")
    prd = nc.dram_tensor("prd", [128, W_CORE * 128], BF16,
                         kind="ExternalInput")
    nodes_t = nc.dram_tensor("nodes_t", [66, NODES_CORE], F32,
                             kind="ExternalInput")
    invdeg = nc.dram_tensor("invdeg", [WN, W_CORE], F32, kind="ExternalInput")
    w2 = nc.dram_tensor("w2", [HIDDEN, OUT_F], BF16, kind="ExternalInput")
    waug = nc.dram_tensor("waug", [66, OUT_F], F32, kind="ExternalInput")
    out_d = nc.dram_tensor("out", [NODES_CORE, OUT_F], F32,
                           kind="ExternalOutput")

    relu = mybir.ActivationFunctionType.Relu
    cpy = mybir.ActivationFunctionType.Copy

    with tile.TileContext(nc) as tc:
        with ExitStack() as ctx:
            cpool = ctx.enter_context(tc.tile_pool(name="const", bufs=1))
            bpool = ctx.enter_context(tc.tile_pool(name="batch", bufs=3))
            empool = ctx.enter_context(tc.tile_pool(name="emp", bufs=4))
            spool = ctx.enter_context(tc.tile_pool(name="small", bufs=4))
            opool = ctx.enter_context(tc.tile_pool(name="outs", bufs=3))
            ph = ctx.enter_context(tc.tile_pool(name="ph", bufs=2, space="PSUM"))
            pagg = ctx.enter_context(tc.tile_pool(name="pagg", bufs=2,
                                                  space="PSUM"))
            pout = ctx.enter_context(tc.tile_pool(name="pout", bufs=1,
                                                  space="PSUM"))
            obpool = ctx.enter_context(tc.tile_pool(name="ob", bufs=2))

            w2_t = cpool.tile([HIDDEN, OUT_F], BF16)
            nc.sync.dma_start(w2_t[:], w2.ap())
            waug_t = cpool.tile([66, OUT_F], F32)
            nc.sync.dma_start(waug_t[:], waug.ap())
            invdeg_t = cpool.tile([WN, W_CORE], F32)
            nc.sync.dma_start(invdeg_t[:], invdeg.ap())
            sidx_t = cpool.tile([128, slots_core // 16], I16)
            nc.sync.dma_start(sidx_t[:], sidx.ap())

            for b in range(NBATCH):
                s0 = b * slots_b                      # batch slot base
                em = empool.tile([128, slots_b], BF16, tag="em")
                fs = bpool.tile([128, slots_b], FP8, tag="fs")
                mk = bpool.tile([128, tiles_b * WN], FP8, tag="mk")
                pr = bpool.tile([128, B * 128], BF16, tag="pr")
                nodesb_t = bpool.tile([66, B * WN], F32, tag="nodesb")
                nc.sync.dma_start(fs[:], fsd.ap()[:, s0:s0 + slots_b])
                nc.sync.dma_start(
                    mk[:], maskd.ap()[:, b * tiles_b * WN:(b + 1) * tiles_b * WN])
                nc.sync.dma_start(pr[:],
                                  prd.ap()[:, b * B * 128:(b + 1) * B * 128])
                nc.sync.dma_start(nodesb_t[:],
                                  nodes_t.ap()[:, b * B * WN:(b + 1) * B * WN])

                # sender gathers: groups 0-3 source quarters on the 4 SWDGE
                # queues; the overflow group's table rows are host-packed in
                # padded slot order, so it loads as one dense strided DMA
                for g in range(4):
                    gs = B * caps[g]
                    nc.gpsimd.dma_gather(
                        out_ap=em[:, goff[g]:goff[g] + gs]
                        .rearrange("p (c f) -> p c f", f=128),
                        in_ap=tbl_s.ap()[g * QUARTER:(g + 1) * QUARTER, :],
                        idxs_ap=sidx_t[:, (s0 + goff[g]) // 16:
                                       (s0 + goff[g] + gs) // 16],
                        num_idxs=gs, num_idxs_reg=gs, elem_size=128,
                        transpose=False, single_packet=False,
                        queue_num=(g + b) % 4,
                    )
                nc.sync.dma_start(
                    out=em[:, goff[4]:goff[4] + B * capo]
                    .rearrange("p (c f) -> p c f", f=128),
                    in_=tbl_s.ap()[NODES_PAD + b * B * capo:
                                   NODES_PAD + (b + 1) * B * capo, :]
                    .rearrange("(c p) f -> p c f", p=128))

                otb = obpool.tile([WN, B * OUT_F], F32, tag="otb")
                for wi in range(B):
                    wg = b * B + wi                   # global window index
                    agg_ps = pagg.tile([128, WN], F32, tag="agg")
                    for pi, pair in enumerate(pairs):
                        h_ps = ph.tile([128, pw], F32, tag="h")
                        hp = spool.tile([128, pw], BF16, tag="hp")
                        pcols = sum(caps[g] for g in pair)
                        for gi, g in enumerate(pair):
                            cg = caps[g]
                            off = goff[g] + wi * cg   # batch-local slot base
                            for j in range(cg // 128):
                                so = off + j * 128
                                nc.tensor.matmul(
                                    out=h_ps[:, gi * capq + j * 128:
                                             gi * capq + (j + 1) * 128],
                                    lhsT=fs[:, so:so + 128],
                                    rhs=pr[:, wi * 128:(wi + 1) * 128],
                                    start=True, stop=True)
                        if len(pair) == 2:
                            # one wide add for both groups: em view [128,2,capq]
                            # strided by the group-region pitch B*capq
                            a = em[:, goff[pair[0]] + wi * capq:
                                   goff[pair[0]] + wi * capq + capq]
                            em3 = bass.AP(a.tensor, a.offset,
                                          [list(a.ap[0]), [B * capq, 2],
                                           [1, capq]])
                            nc.vector.tensor_add(
                                hp[:, :2 * capq]
                                .rearrange("p (c f) -> p c f", f=capq),
                                em3,
                                h_ps[:, :2 * capq]
                                .rearrange("p (c f) -> p c f", f=capq))
                        else:
                            cg = caps[pair[0]]
                            off = goff[pair[0]] + wi * cg
                            nc.vector.tensor_add(hp[:, :cg],
                                                 em[:, off:off + cg],
                                                 h_ps[:, :cg])
                        hs = spool.tile([128, pw], BF16, tag="hs")
                        nc.scalar.activation(hs[:, :pcols], hp[:, :pcols], relu)
                        for gi, g in enumerate(pair):
                            cg = caps[g]
                            off = goff[g] + wi * cg
                            for j in range(cg // 128):
                                tb = (off + j * 128) // 128   # batch tile idx
                                nc.tensor.matmul(
                                    out=agg_ps[:],
                                    lhsT=hs[:, gi * capq + j * 128:
                                            gi * capq + (j + 1) * 128],
                                    rhs=mk[:, tb * WN:(tb + 1) * WN],
                                    start=(pi == 0 and gi == 0 and j == 0),
                                    stop=(pi == 2 and j == cg // 128 - 1))
                    # window epilogue: out = (aggT.T@W2)*invdeg + nodes@waug
                    agg_s = opool.tile([128, WN], BF16, tag="aggs")
                    nc.scalar.activation(agg_s[:], agg_ps[:], cpy)
                    ot_ps = pout.tile([WN, 2 * OUT_F], F32, tag="ot")
                    nc.tensor.matmul(out=ot_ps[:, 0:OUT_F], lhsT=agg_s[:],
                                     rhs=w2_t[:], start=True, stop=True)
                    nc.tensor.matmul(out=ot_ps[:, OUT_F:2 * OUT_F],
                                     lhsT=nodesb_t[:, wi * WN:(wi + 1) * WN],
                                     rhs=waug_t[:], start=True, stop=True)
                    t1 = opool.tile([WN, OUT_F], F32, tag="t1")
                    nc.vector.tensor_scalar(
                        out=t1[:], in0=ot_ps[:, 0:OUT_F],
                        scalar1=invdeg_t[:, wg:wg + 1],
                        scalar2=None, op0=mybir.AluOpType.mult)
                    nc.vector.tensor_add(otb[:, wi * OUT_F:(wi + 1) * OUT_F],
                                         t1[:], ot_ps[:, OUT_F:2 * OUT_F])
                nc.sync.dma_start(
                    out_d.ap()[b * B * WN:(b + 1) * B * WN, :]
                    .rearrange("(w n) f -> n w f", n=WN),
                    otb[:].rearrange("p (w f) -> p w f", f=OUT_F))

    nc.compile()
    return nc


def _prep_inputs(nodes, edges, senders, receivers, W1, b1, W2, b2, Wn, bn,
                 capq, capo):
    """Host-side data layout. Returns per-core in_maps."""
    caps = [capq] * 4 + [capo]
    slots_w = sum(caps)
    slots_core = W_CORE * slots_w
    tiles_core = slots_core // 128
    goff = [0]
    for c in caps:
        goff.append(goff[-1] + B * c)
    slots_b = B * slots_w

    nodes_pad = np.zeros((NODES_PAD, NODE_F), np.float32)
    nodes_pad[:N_NODES] = nodes

    # per-node projections (host): sender table + receiver table (+b1)
    ps_full = (nodes_pad @ W1[:NODE_F]).astype(bfnp)            # [NP, 128]
    pr_full = (nodes_pad @ W1[NODE_F:2 * NODE_F] + b1).astype(bfnp)
    w1e = W1[2 * NODE_F:].astype(bfnp)                          # [32, 128]

    tbl_base = ps_full

    deg = np.bincount(receivers, minlength=NODES_PAD).astype(np.float32)
    invdeg_full = 1.0 / np.maximum(deg, 1.0)
    gate_full = (deg > 0).astype(np.float32)

    w2b = W2.astype(bfnp)
    waug = np.zeros((66, OUT_F), np.float32)
    waug[:NODE_F] = Wn
    waug[NODE_F] = b2
    waug[NODE_F + 1] = bn

    core_of_edge = receivers // NODES_CORE
    in_maps = []
    for c in range(NCORES):
        lo = c * NODES_CORE
        eid = np.nonzero(core_of_edge == c)[0]
        rloc = receivers[eid] - lo
        w_loc = rloc // WN
        q = senders[eid] // QUARTER
        order = np.lexsort((q, w_loc))
        eid, rloc, w_loc, q = eid[order], rloc[order], w_loc[order], q[order]
        grp = w_loc * 4 + q
        starts = np.searchsorted(grp, np.arange(W_CORE * 4))
        pos = np.arange(len(eid)) - starts[grp]

        w_batch = w_loc // B
        w_in = w_loc % B
        normal = pos < capq
        # normal slots: group q region of the window's batch
        goff_a = np.array(goff[:5])
        slot = np.empty(len(eid), np.int64)
        slot[normal] = (w_batch[normal] * slots_b + goff_a[q[normal]]
                        + w_in[normal] * capq + pos[normal])
        sidx_v = np.empty(len(eid), np.int64)
        sidx_v[normal] = senders[eid[normal]] % QUARTER

        # overflow: per-window list, region-5 table rows
        ov = ~normal
        ov_idx = np.nonzero(ov)[0]
        ovw = w_loc[ov_idx]
        ostarts = np.searchsorted(ovw, np.arange(W_CORE))
        opos = np.arange(len(ov_idx)) - ostarts[ovw]
        assert opos.max(initial=0) < capo, \
            f"window overflow {opos.max(initial=0) + 1} > {capo}"
        slot[ov_idx] = (w_batch[ov_idx] * slots_b + goff[4]
                        + w_in[ov_idx] * capo + opos)
        sidx_v[ov_idx] = 0
        tbl_c = np.zeros((NODES_PAD + W_CORE * capo, 128), bfnp)
        tbl_c[:NODES_PAD] = tbl_base[:NODES_PAD]
        tbl_c[NODES_PAD + w_loc[ov_idx] * capo + opos] = \
            tbl_base[senders[eid[ov_idx]]]

        sidx_f = np.zeros(slots_core, np.int16)
        sidx_f[slot] = sidx_v.astype(np.int16)
        nloc = (rloc % WN).astype(np.int64)
        # fused stationary: rows 0:96 maskT, rows 96:128 edge features (fp8)
        fsd_c = np.zeros((128, slots_core), f8np)
        fsd_c[nloc, slot] = 1.0
        fsd_c[WN:, slot] = edges[eid].T.astype(f8np)
        # aggregation mask, 96 cols per 128-slot tile
        maskd_c = np.zeros((128, tiles_core * WN), f8np)
        maskd_c[slot & 127, (slot >> 7) * WN + nloc] = 1.0

        nodes_taug = np.zeros((66, NODES_CORE), np.float32)
        nodes_taug[:NODE_F] = nodes_pad[lo:lo + NODES_CORE].T
        nodes_taug[NODE_F] = gate_full[lo:lo + NODES_CORE]
        nodes_taug[NODE_F + 1] = 1.0

        # moving-operand table: rows 0:96 PR' per window, rows 96:128 W1e
        prd_c = np.zeros((128, W_CORE * 128), bfnp)
        prd_c[:WN] = (pr_full[lo:lo + NODES_CORE].reshape(W_CORE, WN, HIDDEN)
                      .transpose(1, 0, 2).reshape(WN, W_CORE * HIDDEN))
        prd_c[WN:] = np.tile(w1e, (1, W_CORE))

        in_maps.append({
            "tbl_s": tbl_c,
            "fsd": fsd_c,
            "maskd": maskd_c,
            "sidx": np.tile(sidx_f.reshape(-1, 16).T, (8, 1)),
            "prd": prd_c,
            "nodes_t": nodes_taug,
            "invdeg": invdeg_full[lo:lo + NODES_CORE].reshape(W_CORE, WN).T.copy(),
            "w2": w2b, "waug": waug,
        })
    return in_maps


def kernel(nodes, edges, senders, receivers, W1, b1, W2, b2, Wn, bn,
           _trace=False):
    senders = np.asarray(senders).astype(np.int64)
    receivers = np.asarray(receivers).astype(np.int64)
    nodes = np.asarray(nodes, np.float32)
    edges = np.asarray(edges, np.float32)

    # capacities; grow if the data exceeds them (recompile)
    capq, capo = 384, 128
    cw = (receivers // NODES_CORE) * (W_CORE * 4) \
        + ((receivers % NODES_CORE) // WN) * 4 + senders // QUARTER
    cnt = np.bincount(cw, minlength=NCORES * W_CORE * 4)
    ovw = np.maximum(cnt - capq, 0).reshape(-1, 4).sum(1)
    while ovw.max() > capo:
        capo += 128
    if np.maximum(cnt - capq, 0).reshape(NCORES, -1).sum(1).max() > OVR:
        capq += 128  # pathological: shrink overflow volume instead

    key = (capq, capo)
    if key not in _cache:
        _cache[key] = _build_program(capq, capo)
    nc = _cache[key]

    in_maps = _prep_inputs(nodes, edges, senders, receivers,
                           np.asarray(W1, np.float32), np.asarray(b1, np.float32),
                           np.asarray(W2, np.float32), np.asarray(b2, np.float32),
                           np.asarray(Wn, np.float32), np.asarray(bn, np.float32),
                           capq, capo)

    res = bass_utils.run_bass_kernel_spmd(
        nc, in_maps, core_ids=list(range(NCORES)), trace=_trace)

    out = np.concatenate([res.results[c]["out"] for c in range(NCORES)], axis=0)
    kernel.last_results = res
    return out[:N_NODES]
